# revision 23
# baseline (speedup 1.0000x reference)
"""Trainium2 Bass kernel for nn_Luong_61684320305412 (bidirectional masked
softmax attention, B=8, L0=L1=2048, D=256), data-parallel over batch
(one batch element per NeuronCore).  ~122us vs the 274us v1 baseline.

Math per core (F = exp(S/256) elementwise on raw scores S = q0 @ q1^T):
    E    = F * (1 - m0 x m1)         (outer-product mask -> exact zeros)
    out0 = (E @ q1) / 16 / rowsum(E)
    out1 = (E^T @ q0) / 16 / colsum(E)

Design:
  - Host-side sort: rows (l0) by mask0, cols (l1) by mask1, unmasked first.
    The (masked x masked) block of E is exactly zero, so its score matmuls,
    exps, and out-matmul contributions are skipped (static conservative
    bounds over the 8 batches; outputs un-permuted on the host).  Boundary
    tiles are masked exactly on the DVE with one fused op:
    E *= max(1 - m_col, 1 - m_row[p])  ==  1 - m_row*m_col  for 0/1 masks.
  - q is pre-converted to bf16 and pre-transposed to the device layout on
    the host, halving the input DMA and making it fully linear; the kernel
    only ever consumes bf16/fp8 forms of q.
  - Score matmuls: fp8e4 + DoubleRow (K=256 in one PE pass, ~230ns per
    [128x512] chunk).  Out-phase matmuls bf16 (16-long accumulation chains
    at ~110ns/MM, stream-bound).  q^T tiles built with regular identity
    matmuls (FWL weight loads) after a short PE warmup chain.
  - exp on ScalarE is the pacing engine (~61us, >98% saturated between the
    first and last exp).  Phases are emitted S0, then interleaved (S1 tile,
    out0 chain), then out1, with emission order chosen for each engine's
    FIFO; the S psum pool closes after the last S tile to give the final
    out phase more crawling psum slots.
  - Softmax denominators via ones-columns appended to the bf16 rhs
    (psum col 256 accumulates row/col sums for free).

Known floors: ~7us fixed launch, ~61us ScalarE exp (1 elem/lane/cycle,
dtype-independent; PSUM bank budget blocks larger activation tiles), and a
~20us final-phase tail (PE executes its queue in order, so out-chains for
the second orientation cannot complete before its last exps; k-major
emission was tried and loses more to PSUM accumulation-group cycling than
the tail saves).
"""

from contextlib import ExitStack

import ml_dtypes
import numpy as np

import concourse.bass as bass
import concourse.tile as tile
from concourse import bacc, mybir
from concourse.bass_utils import run_bass_kernel_spmd
from concourse.masks import make_identity

P = 128
B = 8
L = 2048
D = 256
T = L // P
DC = D // P
AUGW = D + 2
SCALE2 = 1.0 / 256.0
SCALE1 = 1.0 / 16.0

f32 = mybir.dt.float32
f32r = mybir.dt.float32r
bf16 = mybir.dt.bfloat16
f8e4 = mybir.dt.float8e4
i32 = mybir.dt.int32
MUL = mybir.AluOpType.mult
ADD = mybir.AluOpType.add
MAX = mybir.AluOpType.max
EXP = mybir.ActivationFunctionType.Exp
DR = mybir.MatmulPerfMode.DoubleRow

USE_FP8_SCORES = True


def _emit(tc: tile.TileContext, ctx: ExitStack, io: dict, cfg: dict):
    nc = tc.nc
    q0, q1, m0, m1 = io["q0"], io["q1"], io["mask0"], io["mask1"]
    out0, out1 = io["out0"], io["out1"]

    RT = (cfg["rt0"], cfg["rt1"])
    CT = (cfg["rt1"], cfg["rt0"])
    TRB = (cfg["trb0"], cfg["trb1"])
    CMIN = (cfg["cmin1"], cfg["cmin0"])

    consts = ctx.enter_context(tc.tile_pool(name="consts", bufs=1))
    qaug = ctx.enter_context(tc.tile_pool(name="qaug", bufs=1))
    qT = ctx.enter_context(tc.tile_pool(name="qT", bufs=1))

    # ---- constants (no DMA deps) ----
    ident_f = consts.tile([P, P], f32)
    make_identity(nc, ident_f)
    identb = consts.tile([P, P], bf16)
    nc.vector.tensor_copy(out=identb, in_=ident_f)
    # per-row complement columns
    mc1m = []
    for idx, msrc in enumerate((m0, m1)):
        mci = consts.tile([P, T], i32, name=f"mci{idx}")
        nc.sync.dma_start(out=mci, in_=msrc.rearrange("(t p) -> p t", p=P))
        mcf = consts.tile([P, T], f32, name=f"mcf{idx}")
        nc.vector.tensor_copy(out=mcf, in_=mci)
        mcn = consts.tile([P, T], f32, name=f"mcn{idx}")
        nc.vector.tensor_scalar(
            out=mcn, in0=mcf, scalar1=-1.0, scalar2=1.0, op0=MUL, op1=ADD
        )
        mc1m.append(mcn)

    # transposed fp8 score operands loaded FIRST: they are all the S0 phase
    # needs, so the first exps start as soon as these 2MB land
    sdt = f8e4 if USE_FP8_SCORES else bf16
    q0t = qT.tile([P, DC, L], sdt)
    q1t = qT.tile([P, DC, L], sdt)
    HL = L // 2
    for h in (0, 1):
        for name, dst in (("q0t", q0t), ("q1t", q1t)):
            nc.sync.dma_start(
                out=dst[:, :, h * HL : (h + 1) * HL],
                in_=io[name].rearrange("p (c l) -> p c l", c=DC)[
                    :, :, h * HL : (h + 1) * HL
                ],
            )

    # bf16 augmented q (out-phase rhs; not needed until the first out chains)
    q0a = qaug.tile([P, T, AUGW], bf16)
    q1a = qaug.tile([P, T, AUGW], bf16)
    for qsrc, dst in ((q0, q0a), (q1, q1a)):
        nc.sync.dma_start(
            out=dst[:, :, 0:D], in_=qsrc.rearrange("p (t d) -> p t d", t=T)
        )
    ones_f = consts.tile([P, T, 2], f32)
    nc.vector.memset(ones_f, 1.0)
    wzb_f = consts.tile([P, 512], f32)
    nc.vector.memset(wzb_f, 0.0)
    wzb = consts.tile([P, 512], bf16)
    nc.vector.tensor_copy(out=wzb, in_=wzb_f)

    # complement-mask broadcasts (1 - m) come precomputed from the host
    mbc = []
    for idx, msrc in enumerate((io["mbc0"], io["mbc1"])):
        mb = qaug.tile([P, L], bf16, name=f"mbc{idx}")
        nc.sync.dma_start(out=mb, in_=msrc)
        mbc.append(mb)

    for dst in (q0a, q1a):
        nc.vector.tensor_copy(out=dst[:, :, D:AUGW], in_=ones_f)

    e_pool = ctx.enter_context(tc.tile_pool(name="e", bufs=52))
    outp = ctx.enter_context(tc.tile_pool(name="outp", bufs=4))
    small = ctx.enter_context(tc.tile_pool(name="small", bufs=4))
    o_psum = ctx.enter_context(tc.tile_pool(name="o_psum", bufs=2, space="PSUM"))
    s_psum = []

    HW = 1024  # etile column-half width
    ehalves = [[], []]

    def emit_s_tile(orient, t):
        lT, rT = (q0t, q1t) if orient == 0 else (q1t, q0t)
        lm, rm = (0, 1) if orient == 0 else (1, 0)
        rt, ct, trb, cmin = RT[orient], CT[orient], TRB[orient], CMIN[orient]
        eh = [
            e_pool.tile([P, HW], bf16, tag="E", name=f"e{orient}_{t}_{h}")
            for h in range(2)
        ]
        ehalves[orient].append(eh)
        ncols = L if t < rt else ct * P
        offs = []
        off = 0
        while off < ncols:
            w = min(512, ncols - off)
            offs.append((off, w))
            off += w
        for pi in range(0, len(offs), 2):
            pair = offs[pi : pi + 2]
            pw = sum(w for _, w in pair)
            ps = s_psum[0].tile([P, 1024], f32, tag="sp")
            base = pair[0][0]
            for off, w in pair:
                sl = ps[:, off - base : off - base + w]
                if USE_FP8_SCORES:
                    nc.tensor.matmul(
                        sl,
                        lhsT=lT[:, :, t * P : (t + 1) * P],
                        rhs=rT[:, :, off : off + w],
                        start=True,
                        stop=True,
                        perf_mode=DR,
                    )
                else:
                    for dc in range(DC):
                        nc.tensor.matmul(
                            sl,
                            lhsT=lT[:, dc, t * P : (t + 1) * P],
                            rhs=rT[:, dc, off : off + w],
                            start=(dc == 0),
                            stop=(dc == DC - 1),
                        )
            nc.scalar.activation(
                out=eh[pi // 2][:, 0:pw], in_=ps[:, 0:pw], func=EXP, scale=SCALE2
            )
        a = (cmin // 2) * 2
        b = ncols
        if t >= trb and b > a:
            # E *= max(1 - m_col, 1 - m_row[p])  == 1 - m_row*m_col
            for h in range(2):
                ha, hb = max(a, h * HW), min(b, (h + 1) * HW)
                if hb > ha:
                    nc.vector.scalar_tensor_tensor(
                        out=eh[h][:, ha - h * HW : hb - h * HW],
                        in0=mbc[rm][:, ha:hb],
                        scalar=mc1m[lm][:, t : t + 1],
                        in1=eh[h][:, ha - h * HW : hb - h * HW],
                        op0=MAX,
                        op1=MUL,
                    )

    def emit_out_chain(orient, mt, opool):
        raug = q0a if orient == 0 else q1a
        odram = out1 if orient == 0 else out0
        rt, ct = RT[orient], CT[orient]
        kmax = T if mt < ct else rt
        h, hoff = mt // 8, (mt % 8) * P
        po = opool.tile([P, AUGW], f32, tag="op")
        for k in range(kmax):
            nc.tensor.matmul(
                po,
                lhsT=ehalves[orient][k][h][:, hoff : hoff + P],
                rhs=raug[:, k, :],
                start=(k == 0),
                stop=(k == kmax - 1),
            )
        rc = small.tile([P, 1], f32, tag="rc")
        nc.vector.reciprocal(rc, po[:, D : D + 1])
        ot = outp.tile([P, D], f32, tag="ot")
        nc.vector.tensor_scalar(
            out=ot, in0=po[:, 0:D], scalar1=rc, scalar2=SCALE1, op0=MUL, op1=MUL
        )
        nc.sync.dma_start(out=odram[mt * P : (mt + 1) * P, :], in_=ot)

    # S0 fully; then alternate (S1 tile, out0 chain) so or1's exps overlap
    # out0's PE time.  The S psum pool closes after the last S tile, freeing
    # 6 banks for the final out phase: 8 chains crawl behind the late exps
    # instead of 2, shrinking the post-exp PE tail.
    with tc.tile_pool(name="s_psum", bufs=3, space="PSUM") as s_psum_pool:
        s_psum.append(s_psum_pool)
        for t in range(T):
            emit_s_tile(0, t)
        for i in range(T):
            emit_s_tile(1, i)
            emit_out_chain(0, i, o_psum)
    with tc.tile_pool(name="o2_psum", bufs=6, space="PSUM") as o2_psum:
        for mt in range(T):
            emit_out_chain(1, mt, o2_psum if mt % 4 != 3 else o_psum)


_CACHE = {}


def _build(cfg_key):
    if cfg_key in _CACHE:
        return _CACHE[cfg_key]
    cfg = dict(zip(("rt0", "rt1", "trb0", "trb1", "cmin0", "cmin1"), cfg_key))
    nc = bacc.Bacc("TRN2", target_bir_lowering=False, debug=False)
    io = {
        "q0": nc.dram_tensor("q0", [P, T * D], bf16, kind="ExternalInput").ap(),
        "q1": nc.dram_tensor("q1", [P, T * D], bf16, kind="ExternalInput").ap(),
        "mask0": nc.dram_tensor("mask0", [L], i32, kind="ExternalInput").ap(),
        "mask1": nc.dram_tensor("mask1", [L], i32, kind="ExternalInput").ap(),
        "mbc0": nc.dram_tensor("mbc0", [P, L], bf16, kind="ExternalInput").ap(),
        "mbc1": nc.dram_tensor("mbc1", [P, L], bf16, kind="ExternalInput").ap(),
        "q0t": nc.dram_tensor("q0t", [P, DC * L], f8e4, kind="ExternalInput").ap(),
        "q1t": nc.dram_tensor("q1t", [P, DC * L], f8e4, kind="ExternalInput").ap(),
        "out0": nc.dram_tensor("out0", [L, D], f32, kind="ExternalOutput").ap(),
        "out1": nc.dram_tensor("out1", [L, D], f32, kind="ExternalOutput").ap(),
    }
    with tile.TileContext(nc) as tc:
        with ExitStack() as ctx:
            _emit(tc, ctx, io, cfg)
    nc.compile()
    _CACHE[cfg_key] = nc
    return nc


def _dev_layout_t(qs):
    """[L, D] fp32 -> fp8 [128, DC*L]: element (p, c*L+l) = q[l, c*128+p]."""
    arr = qs.T.reshape(DC, P, L).transpose(1, 0, 2).reshape(P, DC * L)
    return np.ascontiguousarray(arr).astype(ml_dtypes.float8_e4m3fn)


def _dev_layout(qs):
    """[L, D] fp32 -> bf16 [128, T*D]: partition p holds rows p, 128+p, ..."""
    return np.ascontiguousarray(
        qs.reshape(T, P, D).transpose(1, 0, 2).reshape(P, T * D)
    ).astype(ml_dtypes.bfloat16)


def run_on_cores(q0, q1, mask0, mask1, trace=False):
    q0 = np.asarray(q0, dtype=np.float32)
    q1 = np.asarray(q1, dtype=np.float32)
    mask0 = np.asarray(mask0, dtype=np.int32)
    mask1 = np.asarray(mask1, dtype=np.int32)

    perm0 = [np.argsort(mask0[b], kind="stable") for b in range(B)]
    perm1 = [np.argsort(mask1[b], kind="stable") for b in range(B)]
    r0 = np.array([int((mask0[b] == 0).sum()) for b in range(B)])
    c0 = np.array([int((mask1[b] == 0).sum()) for b in range(B)])

    rt0 = max(1, min(T, -(-int(r0.max()) // P)))
    rt1 = max(1, min(T, -(-int(c0.max()) // P)))
    trb0 = int(r0.min()) // P
    trb1 = int(c0.min()) // P
    cmin0 = int(r0.min())
    cmin1 = int(c0.min())
    cfg_key = (rt0, rt1, trb0, trb1, cmin0, cmin1)

    nc = _build(cfg_key)
    in_maps = []
    for b in range(B):
        in_maps.append(
            {
                "q0": _dev_layout(q0[b][perm0[b]]),
                "q1": _dev_layout(q1[b][perm1[b]]),
                "q0t": _dev_layout_t(q0[b][perm0[b]]),
                "q1t": _dev_layout_t(q1[b][perm1[b]]),
                "mask0": np.ascontiguousarray(mask0[b][perm0[b]]),
                "mask1": np.ascontiguousarray(mask1[b][perm1[b]]),
                "mbc0": np.ascontiguousarray(
                    np.broadcast_to(
                        (1 - mask0[b][perm0[b]]).astype(ml_dtypes.bfloat16), (P, L)
                    )
                ),
                "mbc1": np.ascontiguousarray(
                    np.broadcast_to(
                        (1 - mask1[b][perm1[b]]).astype(ml_dtypes.bfloat16), (P, L)
                    )
                ),
            }
        )
    br = run_bass_kernel_spmd(nc, in_maps, list(range(B)), trace=trace)
    out0 = np.empty((B, L, D), dtype=np.float32)
    out1 = np.empty((B, L, D), dtype=np.float32)
    for b in range(B):
        out0[b][perm0[b]] = br.results[b]["out0"]
        out1[b][perm1[b]] = br.results[b]["out1"]
    return out0, out1, br


def kernel(q0, q1, len0=None, len1=None, mask0=None, mask1=None, **_):
    out0, out1, _br = run_on_cores(q0, q1, mask0, mask1, trace=False)
    return out0, out1
